# revision 3
# baseline (speedup 1.0000x reference)
"""Int4 quantized linear (y = x @ dequant(packed, scale).T + bias) on 8 Trainium2
cores, fp8(e4m3)/fp16 k-split hybrid.

Sharding: column-parallel on out_features (11008 = 8 x 1376). Each core gets the
full activation x and a 1376-row shard of packed/scale/bias, computes its
y[:, shard] for all 8192 tokens; host concatenates shards on the feature axis.

Per-core kernel (contraction 4096 = 32 k-tiles of 128):
  1. Dequant on device: q = (nibble(packed) - 7), NO scale fold (q*scale is not
     exact in fp8). First F8=18 k-tiles -> fp8 e4m3 cache (q in [-7,8] is exact
     in e4m3); remaining 14 -> fp16 cache (also exact). The k order is permuted
     (even/odd nibble unroll); x is host-permuted identically.
  2. Matmul per 128-token subtile: 3 psum groups (out 512+512+352) accumulate in
     flight; consecutive matmuls over the 3 groups share one stationary-x load
     (LDWEIGHTS amortized 3x). fp8 tiles go 2-per-instruction via DoubleRow
     (~1.8x the fp16 rate); fp16 tiles are one matmul each.
  3. Epilogue: y = psum * scale + bias (two DVE ops), DMA to DRAM.

Accuracy: x is e4m3-quantized on 18/32 of the contraction -> rel err 1.912e-2
measured on hardware for the seed-0 problem (threshold 2e-2); weights are
exact in both halves, accumulation is fp32.
"""

import numpy as np
import ml_dtypes

P = 128
OUT, IN = 11008, 4096
B, S = 4, 2048
TOK = B * S
NCORES = 8
F8 = 18                   # k-tiles (of 32) computed in fp8 via DoubleRow

_PROGRAM_CACHE = {}


def _splits(total, step):
    return [(s, min(step, total - s)) for s in range(0, total, step)]


def build_program(tok=TOK, in_dim=IN, out_sh=OUT // NCORES, m_tile=512,
                  n_tile=512, f8=F8, n_reps=1):
    """Build and compile the per-core Bass program. Returns (nc, f8).

    n_reps > 1 wraps the compute in a For_i hardware loop (benchmark builds
    only; the result is identical since y is fully rewritten each pass).
    """
    import contextlib
    import concourse.bacc as bacc
    import concourse.mybir as mybir
    import concourse.tile as tile

    dt = mybir.dt
    alu = mybir.AluOpType
    DR = mybir.MatmulPerfMode.DoubleRow

    ko_n = in_dim // P
    nh = in_dim // 2 // P
    ko16 = ko_n - f8
    assert f8 % 2 == 0 and 0 < f8 < ko_n
    msub = m_tile // P
    m_tiles = _splits(tok, m_tile)
    n_tiles = _splits(out_sh, n_tile)

    nc = bacc.Bacc("TRN2", target_bir_lowering=False, debug=False,
                   num_devices=NCORES)

    x8 = nc.dram_tensor("x8", [P, f8, tok], dt.float8e4, kind="ExternalInput").ap()
    x16 = nc.dram_tensor("x16", [P, ko16, tok], dt.float16, kind="ExternalInput").ap()
    pk3 = nc.dram_tensor("pk3", [P, nh, out_sh], dt.int16, kind="ExternalInput").ap()
    scale_bc = nc.dram_tensor("scale_bc", [P, out_sh], dt.float32, kind="ExternalInput").ap()
    bias_bc = nc.dram_tensor("bias_bc", [P, out_sh], dt.float32, kind="ExternalInput").ap()
    y = nc.dram_tensor("y", [tok, out_sh], dt.float32, kind="ExternalOutput").ap()

    with tile.TileContext(nc) as tc:
        with tc.tile_pool(name="const", bufs=1) as cpool, \
             tc.tile_pool(name="wcache", bufs=1) as wpool, \
             tc.tile_pool(name="pkpool", bufs=3) as pkpool, \
             tc.tile_pool(name="deq", bufs=3) as dqpool, \
             tc.tile_pool(name="xin", bufs=2) as xpool, \
             tc.tile_pool(name="yout", bufs=4) as ypool, \
             tc.tile_pool(name="psum", bufs=8, space="PSUM") as pspool, \
             (tc.For_i(0, n_reps, 1) if n_reps > 1
              else contextlib.nullcontext()):

            scale_t = cpool.tile([P, out_sh], dt.float32)
            nc.sync.dma_start(out=scale_t[:], in_=scale_bc)
            bias_t = cpool.tile([P, out_sh], dt.float32)
            nc.sync.dma_start(out=bias_t[:], in_=bias_bc)

            # --- dequant: q = nibble - 7, exact in fp8/fp16; no scale fold ---
            wT8 = wpool.tile([P, f8, out_sh], dt.float8e4, name="wT8")
            wT16 = wpool.tile([P, ko16, out_sh], dt.float16, name="wT16")
            for h in range(nh):
                pk = pkpool.tile([P, out_sh], dt.int16, name="pk")
                nc.sync.dma_start(out=pk[:], in_=pk3[:, h, :])
                for lo in range(2):
                    ko = 2 * h + lo
                    q = dqpool.tile([P, out_sh], dt.int16, name="q")
                    if lo == 0:
                        nc.vector.tensor_scalar(
                            q[:], pk[:], 15, None, alu.bitwise_and)
                    else:
                        nc.vector.tensor_scalar(
                            q[:], pk[:], 4, None, alu.logical_shift_right)
                    dst = wT8[:, ko, :] if ko < f8 else wT16[:, ko - f8, :]
                    nc.vector.tensor_scalar(dst, q[:], 7, None, alu.subtract)

            # --- matmul + epilogue ---
            for (m0, mlen) in m_tiles:
                xt8 = xpool.tile([P, f8, m_tile], dt.float8e4, name="xt8")
                nc.sync.dma_start(out=xt8[:, :, :mlen], in_=x8[:, :, m0:m0 + mlen])
                xt16 = xpool.tile([P, ko16, m_tile], dt.float16, name="xt16")
                nc.sync.dma_start(out=xt16[:, :, :mlen], in_=x16[:, :, m0:m0 + mlen])
                for ms in range(msub):
                    if ms * P >= mlen:
                        break
                    pss = [pspool.tile([P, n_tile], dt.float32, name="ps")[:, :fd]
                           for (n0, fd) in n_tiles]
                    for j in range(f8 // 2):
                        for ti, (n0, fd) in enumerate(n_tiles):
                            nc.tensor.matmul(
                                pss[ti],
                                lhsT=xt8[:, 2 * j:2 * j + 2, ms * P:(ms + 1) * P],
                                rhs=wT8[:, 2 * j:2 * j + 2, n0:n0 + fd],
                                start=(j == 0),
                                stop=False,
                                perf_mode=DR,
                            )
                    for k2 in range(ko16):
                        for ti, (n0, fd) in enumerate(n_tiles):
                            nc.tensor.matmul(
                                pss[ti],
                                lhsT=xt16[:, k2, ms * P:(ms + 1) * P],
                                rhs=wT16[:, k2, n0:n0 + fd],
                                start=False,
                                stop=(k2 == ko16 - 1),
                            )
                    for ti, (n0, fd) in enumerate(n_tiles):
                        yt = ypool.tile([P, n_tile], dt.float32, name="yt")[:, :fd]
                        nc.vector.tensor_mul(
                            out=yt, in0=pss[ti], in1=scale_t[:, n0:n0 + fd])
                        nc.vector.tensor_add(
                            out=yt, in0=yt, in1=bias_t[:, n0:n0 + fd])
                        nc.sync.dma_start(
                            out=y[m0 + ms * P:m0 + (ms + 1) * P, n0:n0 + fd],
                            in_=yt)

    nc.compile()
    return nc, f8


def host_prep_x(x, f8=F8, tok=TOK, in_dim=IN):
    """[B,S,in] fp32 -> x8 [P,F,tok] e4m3 + x16 [P,32-F,tok] fp16, k-permuted.

    Device k-tile ko=2h+lo at partition p takes input index h*256 + 2p + lo
    (even/odd nibble interleave); host permutes x identically so the dot
    product is unchanged. fp8 covers input indices [0, 128*F).
    """
    xf = np.ascontiguousarray(x, dtype=np.float32).reshape(tok, in_dim)
    cut = f8 * P
    ko16 = in_dim // P - f8
    xs = xf[:, :cut].astype(ml_dtypes.float8_e4m3)
    x4 = xs.reshape(tok, f8 // 2, P, 2)
    x8 = np.ascontiguousarray(x4.transpose(2, 1, 3, 0)).reshape(P, f8, tok)
    xs = xf[:, cut:].astype(np.float16)
    x4 = xs.reshape(tok, ko16 // 2, P, 2)
    x16 = np.ascontiguousarray(x4.transpose(2, 1, 3, 0)).reshape(P, ko16, tok)
    return {"x8": x8, "x16": x16}


def host_prep_shard(packed, scale, bias, out_sh, in_dim=IN):
    """Per-core shard prep. packed [out_sh, in//2] int32 -> [128, nh, out_sh] int16."""
    nh = in_dim // 2 // P
    pk = np.asarray(packed, dtype=np.int16)           # values 0..255, exact
    pk3 = np.ascontiguousarray(pk.T.reshape(nh, P, out_sh).transpose(1, 0, 2))
    sc = np.ascontiguousarray(
        np.broadcast_to(np.asarray(scale, np.float32), (P, out_sh)))
    bi = np.ascontiguousarray(
        np.broadcast_to(np.asarray(bias, np.float32), (P, out_sh)))
    return pk3, sc, bi


def make_in_maps(x, packed, scale, bias, f8=F8, ncores=NCORES):
    out_sh = packed.shape[0] // ncores
    xparts = host_prep_x(x, f8=f8)
    in_maps = []
    for c in range(ncores):
        lo, hi = c * out_sh, (c + 1) * out_sh
        pk3, sc, bi = host_prep_shard(packed[lo:hi], scale[lo:hi], bias[lo:hi], out_sh)
        m = dict(xparts)
        m.update({"pk3": pk3, "scale_bc": sc, "bias_bc": bi})
        in_maps.append(m)
    return in_maps


def reference_host(x, packed, scale, bias):
    """Numpy reference (for testing only)."""
    q0 = packed & 15
    q1 = (packed >> 4) & 15
    q = np.stack([q0, q1], axis=-1).reshape(packed.shape[0], -1) - 7
    w = q.astype(np.float32) * np.asarray(scale, np.float32)[:, None]
    xf = np.asarray(x, np.float32).reshape(-1, w.shape[1])
    return (xf @ w.T + np.asarray(bias, np.float32)).reshape(
        x.shape[0], x.shape[1], -1)


def _get_program():
    key = "full"
    if key not in _PROGRAM_CACHE:
        _PROGRAM_CACHE[key] = build_program()
    return _PROGRAM_CACHE[key]


def run_on_hw(inputs, trace=False, trace_kwargs=None):
    """Run the full-size problem on 8 cores. Returns (y_full, BassKernelResults)."""
    from concourse.bass_utils import run_bass_kernel_spmd

    nc, f8 = _get_program()
    in_maps = make_in_maps(inputs["x"], inputs["packed"], inputs["scale"],
                           inputs["bias"], f8=f8)
    kw = {}
    if trace:
        kw["trace"] = True
        if trace_kwargs:
            kw["trace_kwargs"] = trace_kwargs
    res = run_bass_kernel_spmd(nc, in_maps, core_ids=list(range(NCORES)), **kw)
    y = np.concatenate([res.results[c]["y"] for c in range(NCORES)], axis=1)
    y = np.ascontiguousarray(y.reshape(B, S, OUT), dtype=np.float32)
    return y, res


def kernel(x, packed, scale, bias):
    y, _ = run_on_hw({"x": x, "packed": packed, "scale": scale, "bias": bias})
    return y
